# revision 18
# baseline (speedup 1.0000x reference)
"""Distributed Trainium2 kernel for nn_CONNECT_86964497809993 (TGN-style
GNN message passing: last-event aggregation + GRU memory update + community
incidence matmul), sharded over 8 NeuronCores.

v3 strategy — compact + fp8 DoubleRow + e5m2 weight residuals:
  * Host does the index-only routing ('last' aggregation via stable-sort
    scatter) and COMPACTS to the ~63k nodes that received a message;
    untouched nodes copy through on the host and their community
    contribution (incidence[~has].T @ memory[~has]) is pure-input
    preprocessing folded in on the host. Each core gets ceil(K/8) nodes
    padded to whole 256-node blocks.
  * GRU matmuls run as fp8e4 DoubleRow (2 k-subtiles per instruction, 0.5
    PE cycles/col). Weight quantization error is systematic across nodes
    (it biases the 63k-node community sums), so each weight stack W is fed
    as fp8e4(W) plus an e5m2 residual W - fp8e4(W) in a second accumulating
    DoubleRow — restoring near-f16 weight accuracy at 2x PE cost but still
    ~2.6x cheaper than f16 matmuls. Stream (per-node) quantization is iid
    and stays plain fp8e4.
  * Time encoding cos(dt*w+b) is host-computed (f64 range reduction) and
    shipped as an fp8 k-subtile; the GRU bias rides a constant-1 lane
    shipped inside a tiny constant block (no on-device memsets).
  * Community matmul: f16 node memories (lhsT) x fp8e3 incidence moving
    operand (mixed-dtype matmul, measured exact on hw; halves the largest
    DMA stream).
  * Gate algebra on (4,2)-tile psum groups (4+2+1 banks of 8): ACT runs
    sigmoid(r) before sigmoid(z) so the DVE r*hn / xn+r*hn chain unblocks
    early; Pool (gpsimd) takes the mem-n subtract and the inc/mem16 SWDGE
    descriptor generation; DVE keeps the psum-coupled ops and z-blend.
  * Nodes are interleaved 2-per-DRAM-row so every DMA moves >=512B chunks.
"""

import numpy as np
import ml_dtypes

from concourse import bacc
import concourse.mybir as mybir
from concourse.tile import TileContext
from concourse.bass_utils import run_bass_kernel_spmd

# Problem shapes (hardcoded per contract).
N, E, C = 100000, 50000, 256
M, D, F, T = 128, 128, 128, 64
NCORES = 8
P = 128
BLK = 2 * P                  # node block: 2 tiles interleaved per dram row

f32 = mybir.dt.float32
f16 = mybir.dt.float16
fp8 = mybir.dt.float8e4      # e4m3: streams + weight hi part (DoubleRow)
fp8r = mybir.dt.float8e5     # e5m2: weight residual (lo part)
fp8i = mybir.dt.float8e3     # e3m4: incidence moving operand
A = mybir.AluOpType
AF = mybir.ActivationFunctionType
DR = mybir.MatmulPerfMode.DoubleRow

NP_FP8 = ml_dtypes.float8_e4m3
NP_FP8R = ml_dtypes.float8_e5m2
NP_FP8I = ml_dtypes.float8_e3m4

_COMPILED = {}  # NT -> compiled Bacc program


def _units(nt):
    """2-tile groups: three rotating 2-bank psum pools give reuse distance
    3, deep enough to hide the sigmoid->rhn->npre psum-release chain."""
    return [(t, 2) for t in range(0, nt, 2)]


def _slabs(n_units):
    """Group-index ranges per DMA slab: small first slab (fast pipeline
    start), then 6-group (12-tile) slabs."""
    if n_units <= 4:
        return [(0, n_units)]
    cuts = [0, 1, 4]
    while cuts[-1] < n_units - 2:
        cuts.append(min(n_units - 2, cuts[-1] + 6))
    cuts.append(n_units)
    return list(zip(cuts[:-1], cuts[1:]))


def _max_w(nt):
    units = _units(nt)
    return max(sum(units[u][1] for u in range(lo, hi)) * P
               for lo, hi in _slabs(len(units)))


def _build_program(nt):
    assert nt % 2 == 0
    S = nt * P
    max_w = _max_w(nt)
    nc = bacc.Bacc("TRN2", target_bir_lowering=False)

    sdf = nc.dram_tensor("sdf", [P, 3, S], fp8, kind="ExternalInput")
    ten = nc.dram_tensor("ten", [T, S], fp8, kind="ExternalInput")
    mem8 = nc.dram_tensor("mem8", [P, S], fp8, kind="ExternalInput")
    mem16 = nc.dram_tensor("mem16", [S // 2, 2 * M], f16, kind="ExternalInput")
    inc = nc.dram_tensor("inc", [S // 2, 2 * C], fp8i, kind="ExternalInput")
    wpk = nc.dram_tensor("wpk", [P, 4608], fp8, kind="ExternalInput")
    cst = nc.dram_tensor("cst", [64, max_w], fp8, kind="ExternalInput")

    om = nc.dram_tensor("om", [S // 2, 2 * M], f16, kind="ExternalOutput")
    ocm = nc.dram_tensor("ocm", [M, C], f32, kind="ExternalOutput")

    units = _units(nt)
    slabs = _slabs(len(units))

    with TileContext(nc) as tc:
        with tc.tile_pool(name="const", bufs=1) as cpool, \
             tc.tile_pool(name="work", bufs=6) as wpool, \
             tc.tile_pool(name="psA", bufs=1, space="PSUM") as psA, \
             tc.tile_pool(name="psB", bufs=1, space="PSUM") as psB, \
             tc.tile_pool(name="psC", bufs=1, space="PSUM") as psC, \
             tc.tile_pool(name="psD", bufs=1, space="PSUM") as psD:

            # All weight stacks ship in ONE DMA (packed bytes), sliced into
            # DoubleRow-shaped views below: [wih | wihl | whr | whrl | whn |
            # whnl] = [1536 | 1536 | 512 | 512 | 256 | 256] bytes/partition.
            wpk_t = cpool.tile([P, 4608], fp8)
            nc.sync.dma_start(wpk_t[:], wpk[:])

            def wview(lo, hi, b, res=False):
                v = wpk_t[:, lo:hi].rearrange("p (a b) -> p a b", b=b)
                return v.bitcast(fp8r) if res else v

            wih01, wih23 = wview(0, 768, 384), wview(768, 1536, 384)
            wihl01 = wview(1536, 2304, 384, True)
            wihl23 = wview(2304, 3072, 384, True)
            whr_v = wview(3072, 3584, 256)
            whrl_v = wview(3584, 4096, 256, True)
            whn_v = wview(4096, 4352, 128)
            whnl_v = wview(4352, 4608, 128, True)

            # Stream stacks (lhsT): dims 0..4 = s, d, f, tenc+bias+0, mem8.
            # The tenc subtile's constant rows 64..127 (one 1.0 bias lane +
            # zeros) ship once per buffer from `cst` via Pool SWDGE.
            sts = []
            for i in range(3):
                st = cpool.tile([P, 5, max_w], fp8, tag=f"st{i}")
                nc.gpsimd.dma_start(st[64:128, 3, :], cst[:])
                sts.append(st)

            pall0 = psA.tile([P, 1, 2, 4 * M], f32)
            pall1 = psB.tile([P, 1, 2, 4 * M], f32)
            pall2 = psC.tile([P, 1, 2, 4 * M], f32)
            palls = [pall0, pall1, pall2]               # 2 banks each
            comm = psD.tile([M, C], f32)                # 1 bank

            # ---- Software-pipelined emission: per iteration i emit
            #   PE:   gate matmuls of group i      (into pallA/pallB)
            #   ACT:  sigmoids of group i-1, tanh of group i-1
            #   DVE:  rhn/npre of group i-1, zd/out of group i-2
            #   Pool: mem-n subtract of group i-1
            #   PE:   comm matmuls of group i-3
            # so every op is data-ready when it reaches its in-order queue
            # head. Groups alternate pallA (4 tiles) / pallB (2 tiles).
            groups = []   # (ut0, g, slab_idx)
            u2slab = {}
            for si, (ulo, uhi) in enumerate(slabs):
                for u in range(ulo, uhi):
                    groups.append((units[u][0], units[u][1], si))
            slab_meta = {}    # si -> dict of tiles + geometry
            state = {}        # per-group stage outputs

            def load_slab(si):
                ulo, uhi = slabs[si]
                t0 = units[ulo][0]
                w = sum(units[u][1] for u in range(ulo, uhi)) * P
                nb = w // BLK
                c0 = t0 * P
                r0 = c0 // 2
                st = sts[si % 3]
                nc.sync.dma_start(st[:, 0:3, 0:w], sdf[:, :, c0:c0 + w])
                nc.sync.dma_start(st[0:T, 3, 0:w], ten[:, c0:c0 + w])
                nc.sync.dma_start(st[:, 4, 0:w], mem8[:, c0:c0 + w])
                mem_s = wpool.tile([P, nb, 2, M], f16, tag="mem")
                nc.sync.dma_start(
                    mem_s[:], mem16[r0:r0 + w // 2].rearrange(
                        "(q p) (b f) -> p q b f", p=P, b=2))
                inc_s = wpool.tile([P, nb, 2, C], fp8i, tag="inc")
                nc.sync.dma_start(
                    inc_s[:], inc[r0:r0 + w // 2].rearrange(
                        "(q p) (b f) -> p q b f", p=P, b=2))
                out_s = wpool.tile([P, nb, 2, M], f16, tag="out")
                slab_meta[si] = dict(t0=t0, w=w, r0=r0, st=st, mem=mem_s,
                                     inc=inc_s, out=out_s)

            def stage_mm(i):
                ut0, g, si = groups[i]
                if si not in slab_meta:
                    load_slab(si)
                sm = slab_meta[si]
                st, t0 = sm["st"], sm["t0"]
                pall = palls[i % 3]
                for j in range(g):
                    t = ut0 + j
                    cs = slice((t - t0) * P, (t - t0 + 1) * P)
                    pj = pall[:, j // 2, j % 2, :]
                    nc.tensor.matmul(pj[:, 0:3 * M], st[:, 0:2, cs],
                                     wih01, start=True,
                                     stop=False, perf_mode=DR)
                    nc.tensor.matmul(pj[:, 0:3 * M], st[:, 2:4, cs],
                                     wih23, start=False,
                                     stop=False, perf_mode=DR)
                    nc.tensor.matmul(pj[:, 0:3 * M], st[:, 0:2, cs],
                                     wihl01, start=False,
                                     stop=False, perf_mode=DR)
                    nc.tensor.matmul(pj[:, 0:3 * M], st[:, 2:4, cs],
                                     wihl23, start=False,
                                     stop=False, perf_mode=DR)
                    nc.tensor.matmul(pj[:, 0:2 * M], st[:, 3:5, cs],
                                     whr_v, start=False, stop=False,
                                     perf_mode=DR)
                    nc.tensor.matmul(pj[:, 0:2 * M], st[:, 3:5, cs],
                                     whrl_v, start=False, stop=True,
                                     perf_mode=DR)
                    nc.tensor.matmul(pj[:, 3 * M:4 * M], st[:, 3:5, cs],
                                     whn_v, start=True, stop=False,
                                     perf_mode=DR)
                    nc.tensor.matmul(pj[:, 3 * M:4 * M], st[:, 3:5, cs],
                                     whnl_v, start=False, stop=True,
                                     perf_mode=DR)

            def stage_act(i):
                ut0, g, si = groups[i]
                sm = slab_meta[si]
                gb = g // 2
                lb = (ut0 - sm["t0"]) // 2
                pall = palls[i % 3]
                gsl = slice(0, gb)
                rz = wpool.tile([P, gb, 2, 2 * M], f16, tag=f"rz{g}")
                nc.scalar.activation(rz[:], pall[:, gsl, :, 0:2 * M],
                                     AF.Sigmoid)
                rhn = wpool.tile([P, gb, 2, M], f16, tag=f"rhn{g}")
                nc.vector.tensor_tensor(rhn[:], rz[:, :, :, 0:M],
                                        pall[:, gsl, :, 3 * M:4 * M], A.mult)
                npre = wpool.tile([P, gb, 2, M], f16, tag=f"npre{g}")
                nc.vector.tensor_tensor(npre[:], pall[:, gsl, :, 2 * M:3 * M],
                                        rhn[:], A.add)
                state[i] = (rz, npre, slice(lb, lb + gb))

            def stage_tanh(i):
                ut0, g, si = groups[i]
                sm = slab_meta[si]
                gb = g // 2
                rz, npre, msl = state[i]
                n_t = wpool.tile([P, gb, 2, M], f16, tag=f"n{g}")
                nc.scalar.activation(n_t[:], npre[:], AF.Tanh)
                d_t = wpool.tile([P, gb, 2, M], f16, tag=f"d{g}")
                nc.gpsimd.tensor_tensor(d_t[:], sm["mem"][:, msl, :, :],
                                        n_t[:], A.subtract)
                state[i] = (rz, n_t, d_t, msl)

            def stage_blend(i):
                ut0, g, si = groups[i]
                sm = slab_meta[si]
                rz, n_t, d_t, msl = state[i]
                zd = wpool.tile([P, g // 2, 2, M], f16, tag=f"zd{g}")
                nc.vector.tensor_tensor(zd[:], rz[:, :, :, M:2 * M],
                                        d_t[:], A.mult)
                nc.vector.tensor_tensor(sm["out"][:, msl, :, :], n_t[:],
                                        zd[:], A.add)

            def stage_comm(i):
                ut0, g, si = groups[i]
                sm = slab_meta[si]
                lb = (ut0 - sm["t0"]) // 2
                for j in range(g):
                    t = ut0 + j
                    blk = lb + j // 2
                    nc.tensor.matmul(comm[:], sm["out"][:, blk, j % 2, :],
                                     sm["inc"][:, blk, j % 2, :],
                                     start=(t == 0), stop=(t == nt - 1))
                state.pop(i, None)
                # last group of its slab -> flush om
                if i + 1 == len(groups) or groups[i + 1][2] != si:
                    r0, w = sm["r0"], sm["w"]
                    nc.sync.dma_start(
                        om[r0:r0 + w // 2].rearrange(
                            "(q p) (b f) -> p q b f", p=P, b=2), sm["out"][:])

            ng = len(groups)
            for i in range(ng + 6):
                if i < ng:
                    stage_mm(i)
                if 1 <= i <= ng:
                    stage_act(i - 1)
                if 2 <= i <= ng + 1:
                    stage_tanh(i - 2)
                if 4 <= i <= ng + 3:
                    stage_blend(i - 4)
                if 5 <= i <= ng + 4:
                    stage_comm(i - 5)

            cm = wpool.tile([M, C], f32, tag="cm")
            nc.scalar.activation(cm[:], comm[:], AF.Copy)
            nc.sync.dma_start(ocm[:], cm[:])

    nc.compile()
    return nc


def _get_program(nt=62):
    if nt not in _COMPILED:
        _COMPILED[nt] = _build_program(nt)
    return _COMPILED[nt]


def kernel(src, dst, t, last_update, event_feat, src_embeds, dst_embeds,
           nodes_memory, incidence, w_time, b_time, W_ih, W_hh, b_ih, b_hh):
    src = np.asarray(src); dst = np.asarray(dst); t = np.asarray(t)
    last_update = np.asarray(last_update)
    event_feat = np.asarray(event_feat, np.float32)
    src_embeds = np.asarray(src_embeds, np.float32)
    dst_embeds = np.asarray(dst_embeds, np.float32)
    nodes_memory = np.asarray(nodes_memory, np.float32)
    incidence = np.asarray(incidence, np.float32)
    w_time = np.asarray(w_time, np.float32); b_time = np.asarray(b_time, np.float32)
    W_ih = np.asarray(W_ih, np.float32); W_hh = np.asarray(W_hh, np.float32)
    b_ih = np.asarray(b_ih, np.float32); b_hh = np.asarray(b_hh, np.float32)

    # ---- Host routing: 'last' aggregation = stable-sort scatter (index-only)
    src_all = np.concatenate([src, dst])
    t_all = np.concatenate([t, t])
    perm = np.argsort(t_all, kind="stable")
    win = np.zeros(N, np.int64)
    win[src_all[perm]] = perm          # last write = newest event per node
    has = np.bincount(src_all, minlength=N) > 0
    nodes = np.nonzero(has)[0]         # compacted node ids (sorted)
    K = nodes.size

    # Per-core padded size: whole 256-node blocks
    Kc = -(-K // NCORES)
    nt = max(2, -(-Kc // BLK) * 2)
    S = nt * P
    nc_prog = _get_program(nt)

    # Winner event rows for the compacted nodes
    wn = win[nodes]
    lt = wn < E
    w0 = np.where(lt, wn, wn - E)
    emb_s = np.where(lt[:, None], src_embeds[w0], dst_embeds[w0])
    emb_d = np.where(lt[:, None], dst_embeds[w0], src_embeds[w0])
    feat = event_feat[w0]

    # Time encoding on host: fp32 arg (reference rounding), f64 cos, fp8 out
    dtw = (t_all[wn] - last_update[nodes]).astype(np.float32)
    x = dtw[:, None] * w_time[None, :] + b_time[None, :]
    tenc = np.cos(x.astype(np.float64)).astype(np.float32)

    memK = nodes_memory[nodes]
    incK = incidence[nodes]

    # Stream-order permutation: tile t=2s+b, col p  <->  compact 256s+2p+b
    ORD = np.arange(S).reshape(-1, P, 2).transpose(0, 2, 1).reshape(-1)

    def pad_rows(a, rows):
        out = np.zeros((rows,) + a.shape[1:], a.dtype)
        out[:a.shape[0]] = a
        return out

    bias_row = (b_ih + b_hh).astype(np.float32)
    Wt_ext = np.zeros((P, 3 * M), np.float32)
    Wt_ext[0:T] = W_ih[2 * D + F:]
    Wt_ext[T] = bias_row
    wih_v = np.stack([W_ih[0:D], W_ih[D:2 * D], W_ih[2 * D:2 * D + F],
                      Wt_ext]).transpose(1, 0, 2)          # [128, 4, 384]
    whr_v = np.zeros((P, 2, 2 * M), np.float32)
    whr_v[:, 1] = W_hh[:, 0:2 * M]
    whn_v = np.zeros((P, 2, M), np.float32)
    whn_v[:, 1] = W_hh[:, 2 * M:]

    def hi_lo(a):
        hi = np.ascontiguousarray(a).astype(NP_FP8)
        lo = (a - hi.astype(np.float32)).astype(NP_FP8R)
        return hi, lo

    wih_8, wih_l = hi_lo(wih_v)
    whr_8, whr_l = hi_lo(whr_v)
    whn_8, whn_l = hi_lo(whn_v)
    wpk_v = np.concatenate([
        wih_8.reshape(P, -1).view(np.uint8), wih_l.reshape(P, -1).view(np.uint8),
        whr_8.reshape(P, -1).view(np.uint8), whr_l.reshape(P, -1).view(np.uint8),
        whn_8.reshape(P, -1).view(np.uint8), whn_l.reshape(P, -1).view(np.uint8),
    ], axis=1).view(NP_FP8)
    cst_v = np.zeros((64, _max_w(nt)), NP_FP8)
    cst_v[0] = np.float32(1.0)        # bias lane (partition 64 of subtile 3)

    in_maps = []
    core_n = []
    for c in range(NCORES):
        lo = c * Kc
        hi = min(K, (c + 1) * Kc)
        n = hi - lo
        core_n.append(n)
        so = ORD.copy()
        valid = so < n
        so = np.where(valid, so, 0)

        def stream(a):  # [n, 128] f32 -> [128, S] fp8 in stream order
            g = a[lo:lo + n][so] * valid[:, None]
            return np.ascontiguousarray(g.T).astype(NP_FP8)

        sdf_v = np.empty((P, 3, S), dtype=NP_FP8)
        sdf_v[:, 0] = stream(emb_s)
        sdf_v[:, 1] = stream(emb_d)
        sdf_v[:, 2] = stream(feat)
        ten_g = tenc[lo:lo + n][so] * valid[:, None]
        ten_v = np.ascontiguousarray(ten_g.T).astype(NP_FP8)
        mem8_v = stream(memK)

        mem16_v = pad_rows(memK[lo:lo + n], S).astype(np.float16)
        inc_v = pad_rows(incK[lo:lo + n], S).astype(NP_FP8I)

        in_maps.append(dict(
            sdf=sdf_v, ten=ten_v, mem8=mem8_v,
            mem16=np.ascontiguousarray(mem16_v.reshape(S // 2, 2 * M)),
            inc=np.ascontiguousarray(inc_v.reshape(S // 2, 2 * C)),
            wpk=wpk_v, cst=cst_v,
        ))

    res = run_bass_kernel_spmd(nc_prog, in_maps, core_ids=list(range(NCORES)))

    out = np.empty((N + C, M), np.float32)
    out[:N] = nodes_memory
    comm = np.zeros((M, C), np.float64)
    for c in range(NCORES):
        n = core_n[c]
        om_c = res.results[c]["om"].reshape(S, M)[:n].astype(np.float32)
        out[nodes[c * Kc:c * Kc + n]] = om_c
        comm += res.results[c]["ocm"]

    rest = incidence[~has].T.astype(np.float32) @ nodes_memory[~has]
    out[N:] = comm.T.astype(np.float32) + rest
    return out
